# revision 9
# baseline (speedup 1.0000x reference)
"""Trainium2 Bass kernel for GNN copy_src -> segment-mean (dst-sharded, 8 cores).

Strategy
--------
- Partition dst nodes (and their incoming edges) across 8 NeuronCores:
  core c owns dst rows [c*6250, (c+1)*6250).
- Host-side "inspector" pass (numpy): bucket each core's edges by
  128-slot dst block, split each block's edges into two runs by src range
  (dma_gather indices are int16, so the 50000-row table is addressed as
  two halves), pad each run to a multiple of 128 with harmless dummy
  edges (src index 0, slot sentinel -1). Per-dst inverse degrees are
  also computed host-side (pure index data) and shipped as an input.
- Device kernel (identical SPMD program on all 8 cores), all-bf16 data
  path with fp32 PSUM accumulation:
  * dma_gather batches of bf16 source rows (256 B each) from the full
    author_emb table in HBM into SBUF.
  * per gather call, build the 0/1 edge->slot matrices H for all its
    groups in ONE DVE op via is_equal(slot_value, iota) with broadcast
    APs; dummy edges give all-zero rows.
  * TensorE bf16 matmuls accumulate H^T @ G (feature sums) in PSUM per
    128-slot block.
  * per block: multiply by the precomputed 1/deg column, DMA the
    [128, 128] result tile to the output shard.
- Host gathers the 8 output shards into the full [50000, 128] output.
"""

import os
import sys

import numpy as np

for _p in ("/opt/trn_rl_repo",):
    if os.path.isdir(_p) and _p not in sys.path:
        sys.path.insert(0, _p)

from concourse import bacc, mybir  # noqa: E402
import concourse.bass as bass  # noqa: E402
import concourse.tile as tile  # noqa: E402
from concourse.bass_utils import run_bass_kernel_spmd  # noqa: E402

N_NODES = 50000
N_EDGES = 600000
D_FEAT = 128
N_CORES = 8
NLOC = N_NODES // N_CORES          # 6250 dst nodes per core
BLK = 128                          # dst slots per PSUM block
NB = (NLOC + BLK - 1) // BLK       # 49 blocks per core
HALF = 32768                       # int16 index limit for dma_gather
CALLG = 8                          # groups (of 128 rows) per dma_gather call
SINGLE_PACKET = True               # >8 groups/call needs single_packet=False
SWDGE_SCRATCH = 16384              # SWDGE descriptor ring: bytes/partition
# 4 SWDGE queues: the gather ucode routes queue q to Q7 core pair q
# (cpu_id/2 == queue_num), so calls on different queues generate their
# descriptors concurrently on different core pairs.
NUM_QUEUES = 4

_cache = {}


def _prepare(src, dst):
    """Inspector pass: group/pad edges per (core, block, src-half).

    Returns per-core device arrays plus the (core-invariant) group layout.
    """
    core = dst // NLOC
    slot = dst % NLOC
    blk = slot // BLK
    srel = (slot % BLK).astype(np.float32)
    half = (src >= HALF).astype(np.int64)

    cnt = np.zeros((N_CORES, NB, 2), dtype=np.int64)
    np.add.at(cnt, (core, blk, half), 1)
    # groups per (block, half): shared across cores so the SPMD program is identical
    g = (cnt + 127) // 128
    g = g.max(axis=0)  # [NB, 2]
    # every block needs at least one matmul to initialize its PSUM tile
    zero_blocks = g.sum(axis=1) == 0
    g[zero_blocks, 0] = 1

    aoff = np.concatenate([[0], np.cumsum(g[:, 0])])  # A-list group offsets per block
    boff = np.concatenate([[0], np.cumsum(g[:, 1])])
    GA, GB = int(aoff[-1]), int(boff[-1])
    G = GA + GB

    # sort edges by (core, blk, half) once; then slice segments
    key = ((core * NB + blk) * 2 + half)
    order = np.argsort(key, kind="stable")
    key_sorted = key[order]
    src_sorted = src[order]
    srel_sorted = srel[order]
    seg_starts = np.searchsorted(key_sorted, np.arange(N_CORES * NB * 2))
    seg_ends = np.searchsorted(key_sorted, np.arange(N_CORES * NB * 2), side="right")

    idx_vals = np.zeros((N_CORES, G, 128), dtype=np.int16)
    slot_vals = np.full((N_CORES, G, 128), -1.0, dtype=np.float32)
    for c in range(N_CORES):
        for b in range(NB):
            for h in range(2):
                s, e = seg_starts[(c * NB + b) * 2 + h], seg_ends[(c * NB + b) * 2 + h]
                n = e - s
                if n == 0:
                    continue
                ng = int(g[b, h])
                g0 = (aoff[b] if h == 0 else GA + boff[b])
                iv = idx_vals[c, g0:g0 + ng].reshape(-1)
                sv = slot_vals[c, g0:g0 + ng].reshape(-1)
                sseg = src_sorted[s:e]
                iv[:n] = (sseg - HALF * h).astype(np.int16)
                sv[:n] = srel_sorted[s:e]

    # wrapped int16 layout for dma_gather: value (g, q) -> [q%16, 8*g + q//16],
    # replicated across the 8 sixteen-partition stripes
    w = idx_vals.reshape(N_CORES, G, 8, 16).transpose(0, 3, 1, 2).reshape(N_CORES, 16, G * 8)
    idxw = np.tile(w, (1, 8, 1))                       # [C, 128, G*8] int16
    # slot values as bf16 (exact for -1..127); [C, 128, G]
    slotw = slot_vals.transpose(0, 2, 1).astype(np.float32)

    # per-core inverse degrees, [C, 128, NB]: invd[c, p, b] = 1/max(deg, 1)
    deg = np.bincount(dst, minlength=N_NODES).astype(np.float64)
    invd = (1.0 / np.maximum(deg, 1.0)).astype(np.float32)
    invdw = np.zeros((N_CORES, 128, NB), dtype=np.float32)
    for c in range(N_CORES):
        block = np.zeros(NB * BLK, dtype=np.float32)
        block[:NLOC] = invd[c * NLOC:(c + 1) * NLOC]
        invdw[c] = block.reshape(NB, BLK).T

    layout = dict(g=g, aoff=aoff, boff=boff, GA=GA, GB=GB)
    return idxw, slotw, invdw, layout


def _build_program(layout):
    g, aoff, boff = layout["g"], layout["aoff"], layout["boff"]
    GA, GB = layout["GA"], layout["GB"]
    G = GA + GB
    f32 = mybir.dt.float32
    bf16 = mybir.dt.bfloat16

    nc = bacc.Bacc("TRN2", target_bir_lowering=False, debug=False,
                   num_devices=N_CORES, dynamic_dma_scratch_size=SWDGE_SCRATCH,
                   num_swdge_queues=NUM_QUEUES)
    # two separate tensors: dma_gather's ucode mishandles nonzero source-AP
    # offsets on HW, so each int16-addressable half gets its own tensor
    embA = nc.dram_tensor("embA", [HALF, D_FEAT], bf16, kind="ExternalInput").ap()
    embB = nc.dram_tensor("embB", [N_NODES - HALF, D_FEAT], bf16, kind="ExternalInput").ap()
    iota = nc.dram_tensor("iota", [128, BLK], bf16, kind="ExternalInput").ap()
    idxw = nc.dram_tensor("idxw", [128, G * 8], mybir.dt.int16, kind="ExternalInput").ap()
    slotw = nc.dram_tensor("slotw", [128, G], bf16, kind="ExternalInput").ap()
    invdw = nc.dram_tensor("invdw", [128, NB], f32, kind="ExternalInput").ap()
    out = nc.dram_tensor("out", [NLOC, D_FEAT], f32, kind="ExternalOutput").ap()

    # per-list gather call ranges: (list, group0, ngroups)
    calls = {0: [], 1: []}
    for lst, total in ((0, GA), (1, GB)):
        g0 = 0
        while g0 < total:
            ncg = min(CALLG, total - g0)
            calls[lst].append((g0, ncg))
            g0 += ncg

    with tile.TileContext(nc) as tc:
        with (
            tc.tile_pool(name="const", bufs=1) as cpool,
            tc.tile_pool(name="gath", bufs=4) as gpool,
            tc.tile_pool(name="hbuf", bufs=4) as hpool,
            tc.tile_pool(name="evict", bufs=3) as epool,
            tc.tile_pool(name="psum", bufs=4, space="PSUM") as ppool,
        ):
            iota_sb = cpool.tile([128, BLK], bf16, tag="iota")
            nc.sync.dma_start(out=iota_sb[:], in_=iota[:])
            idx_sb = cpool.tile([128, G * 8], mybir.dt.int16, tag="idx")
            nc.sync.dma_start(out=idx_sb[:], in_=idxw[:])
            slot_sb = cpool.tile([128, G], bf16, tag="slot")
            nc.sync.dma_start(out=slot_sb[:], in_=slotw[:])
            invd_sb = cpool.tile([128, NB], f32, tag="invd")
            nc.sync.dma_start(out=invd_sb[:], in_=invdw[:])

            # gather-call state per list: [call_index, gather_tile, h_tile]
            cur = {0: [-1, None, None], 1: [-1, None, None]}
            srcs = {0: embA, 1: embB}

            def get_group(lst, gg):
                """Ensure the gather call containing group gg of list lst is
                issued (gather + batched one-hot build); return the
                [128, 128] rhs AP and the [128, BLK] H AP for that group."""
                ci = gg // CALLG
                if cur[lst][0] != ci:
                    g0, ncg = calls[lst][ci]
                    t = gpool.tile([128, CALLG * 128], bf16, tag=f"g{lst}")
                    col0 = (g0 if lst == 0 else GA + g0) * 8
                    nc.gpsimd.dma_gather(
                        out_ap=t[:, :ncg * 128].rearrange("p (n e) -> p n e", e=128),
                        in_ap=srcs[lst],
                        idxs_ap=idx_sb[:, col0:col0 + ncg * 8],
                        num_idxs=ncg * 128,
                        num_idxs_reg=ncg * 128,
                        elem_size=D_FEAT,
                        single_packet=SINGLE_PACKET,
                    )
                    # batched one-hot build for all ncg groups of this call
                    scol0 = (g0 if lst == 0 else GA + g0)
                    h = hpool.tile([128, CALLG * BLK], bf16, tag=f"h{lst}")
                    sv = slot_sb[:, scol0:scol0 + ncg]
                    sv3 = sv.unsqueeze(2).to_broadcast([128, ncg, BLK])
                    io3 = iota_sb[:].unsqueeze(1).to_broadcast([128, ncg, BLK])
                    nc.vector.tensor_tensor(
                        out=h[:, :ncg * BLK].rearrange("p (n e) -> p n e", e=BLK),
                        in0=sv3,
                        in1=io3,
                        op=mybir.AluOpType.is_equal,
                    )
                    cur[lst] = [ci, t, h]
                loc = gg % CALLG
                return (cur[lst][1][:, loc * 128:(loc + 1) * 128],
                        cur[lst][2][:, loc * BLK:(loc + 1) * BLK])

            for b in range(NB):
                groups = [(0, int(aoff[b]) + k) for k in range(int(g[b, 0]))]
                groups += [(1, int(boff[b]) + k) for k in range(int(g[b, 1]))]
                psum_s = ppool.tile([128, BLK], f32, tag="ps")
                last = len(groups) - 1
                for k, (lst, gg) in enumerate(groups):
                    rhs, h = get_group(lst, gg)
                    nc.tensor.matmul(out=psum_s[:], lhsT=h, rhs=rhs,
                                     start=(k == 0), stop=(k == last))
                ot = epool.tile([128, BLK], f32, tag="ot")
                nc.vector.tensor_scalar(
                    out=ot[:], in0=psum_s[:], scalar1=invd_sb[:, b:b + 1],
                    scalar2=None, op0=mybir.AluOpType.mult,
                )
                rows = min(BLK, NLOC - b * BLK)
                nc.sync.dma_start(out=out[b * BLK:b * BLK + rows, :],
                                  in_=ot[:rows, :])

    # Tile's scheduling pass reorders instructions and round-robins SWDGE
    # completion sems over 8 DMASW lanes in FINAL order.  A sem may only ever
    # be incremented from one SWDGE queue (ring-reclaim correctness), so the
    # queue must be a function of the assigned lane: queue = lane % NUM_QUEUES.
    if NUM_QUEUES > 1:
        from concourse.tile_scheduler import PROC_NAME_TO_IDX
        lane_of = {PROC_NAME_TO_IDX[f"DMASW{i}"]: i for i in range(8)}
        fn = nc.m.functions[0]
        insts = [i for b in fn.blocks for i in b.instructions]
        for inst in insts:
            if isinstance(inst, mybir.InstDMAGatherAnt):
                lane = lane_of.get(inst.bass_scheduled_proc)
                assert lane is not None, "gather not on a DMASW lane"
                inst.queue_num = lane % NUM_QUEUES

    nc.compile()
    return nc


def _in_maps(author_emb, src, dst):
    emb = np.ascontiguousarray(np.asarray(author_emb, dtype=np.float32))
    src = np.asarray(src).astype(np.int64)
    dst = np.asarray(dst).astype(np.int64)
    assert emb.shape == (N_NODES, D_FEAT) and src.shape == (N_EDGES,)

    idxw, slotw, invdw, layout = _prepare(src, dst)
    key = (layout["GA"], layout["GB"], layout["g"].tobytes())
    if key not in _cache:
        _cache[key] = _build_program(layout)
    nc = _cache[key]

    import ml_dtypes
    embh = emb.astype(ml_dtypes.bfloat16)
    iota_np = np.broadcast_to(np.arange(BLK, dtype=np.float32), (128, BLK)).astype(ml_dtypes.bfloat16)
    embA = np.ascontiguousarray(embh[:HALF])
    embB = np.ascontiguousarray(embh[HALF:])
    maps = [
        {"embA": embA, "embB": embB, "iota": np.ascontiguousarray(iota_np),
         "idxw": idxw[c], "slotw": slotw[c].astype(ml_dtypes.bfloat16),
         "invdw": invdw[c]}
        for c in range(N_CORES)
    ]
    return nc, maps


def kernel(author_emb, src, dst, n_nodes):
    nc, maps = _in_maps(author_emb, src, dst)
    res = run_bass_kernel_spmd(nc, maps, list(range(N_CORES)))
    out = np.empty((N_NODES, D_FEAT), dtype=np.float32)
    for c in range(N_CORES):
        out[c * NLOC:(c + 1) * NLOC] = res.results[c]["out"]
    return out


# revision 19
# speedup vs baseline: 3.8020x; 3.8020x over previous
"""Trainium2 Bass kernel for GNN copy_src -> segment-mean (dst-sharded, 8 cores).

Strategy
--------
- Partition dst nodes (and their incoming edges) across 8 NeuronCores:
  core c owns dst rows [c*6250, (c+1)*6250).
- Host-side "inspector" pass (numpy): bucket each core's edges by
  128-slot dst block, split each block's edges into two runs by src range
  (dma_gather indices are int16, so the 50000-row table is addressed as
  two halves), pad each run to a multiple of 128.  Padding is -1 where
  possible (the gather ucode trims trailing negative indices before
  generating descriptors, so padding costs no Q7 time); the first few
  calls use index 0 instead so their SBUF tiles get fully written before
  any stale data could be read.  Per-dst inverse degrees are computed
  host-side (pure index data) and shipped as an input.
- Device kernel (identical SPMD program on all 8 cores), all-bf16 data
  path with fp32 PSUM accumulation:
  * one dma_gather per (dst-block, table-half) segment pulls that
    segment's bf16 source rows (256 B each) from HBM into SBUF.  Gathers
    are spread over all 4 SWDGE queues (the ucode routes queue q to Q7
    core pair q), so up to 4 calls generate descriptors concurrently.
  * per gather call, build the 0/1 edge->slot matrices H for all its
    groups in ONE DVE op via is_equal(slot_value, iota) with broadcast
    APs; padded edges give all-zero rows.
  * TensorE bf16 matmuls accumulate H^T @ G (feature sums) in PSUM per
    128-slot block.
  * per block: multiply by the precomputed 1/deg column, DMA the
    [128, 128] result tile to the output shard.
- Host gathers the 8 output shards into the full [50000, 128] output.
"""

import os
import sys

import numpy as np

for _p in ("/opt/trn_rl_repo",):
    if os.path.isdir(_p) and _p not in sys.path:
        sys.path.insert(0, _p)

from concourse import bacc, mybir  # noqa: E402
import concourse.bass as bass  # noqa: E402
import concourse.tile as tile  # noqa: E402
from concourse.bass_utils import run_bass_kernel_spmd  # noqa: E402

N_NODES = 50000
N_EDGES = 600000
D_FEAT = 128
N_CORES = 8
NLOC = N_NODES // N_CORES          # 6250 dst nodes per core
BLK = 128                          # dst slots per PSUM block
NB = (NLOC + BLK - 1) // BLK       # 49 blocks per core
HALF = 32768                       # int16 index limit for dma_gather
SWDGE_SCRATCH = 16384              # SWDGE descriptor ring: bytes/partition
NUM_QUEUES = 4                     # gather ucode: queue q -> Q7 core pair q
GBUFS = 6                          # gather/h tile pool depth
MAXG = 16                          # max groups per gather call (ring capacity)
NCHUNK = 4                         # idx table load split (startup overlap)
# -1 trailing padding is trimmed by the gather ucode, but with a static
# num_idxs_reg the decode-side ring bookkeeping (descs reserved from the reg
# value) desyncs from the Q7's actual pushed count -> device-wedging DMA
# corruption (observed on HW; CoreSim's reg==valid-count assert is the same
# protocol).  Keep False unless num_idxs_reg is a per-core runtime register.
PAD_NEG = False

_cache = {}


def _segments(g):
    """Call list: one dma_gather per (block, half) segment, split at MAXG.

    Returns list of dicts with keys: lst, b, g0 (group offset within its
    list), ncg, first (True for the first sub-call of the segment; the
    trailing -1 trim only applies to the last sub-call = segment tail).
    """
    aoff = np.concatenate([[0], np.cumsum(g[:, 0])])
    boff = np.concatenate([[0], np.cumsum(g[:, 1])])
    calls = []
    for b in range(g.shape[0]):
        for lst, off in ((0, aoff), (1, boff)):
            total = int(g[b, lst])
            s = 0
            while s < total:
                n = min(MAXG, total - s)
                calls.append(dict(lst=lst, b=b, g0=int(off[b]) + s, ncg=n,
                                  tail=(s + n == total)))
                s += n
    return calls


def _prepare(src, dst):
    """Inspector pass: group/pad edges per (core, block, src-half)."""
    core = dst // NLOC
    slot = dst % NLOC
    blk = slot // BLK
    srel = (slot % BLK).astype(np.float32)
    half = (src >= HALF).astype(np.int64)

    cnt = np.zeros((N_CORES, NB, 2), dtype=np.int64)
    np.add.at(cnt, (core, blk, half), 1)
    # groups per (block, half): shared across cores so the SPMD program is identical
    g = (cnt + 127) // 128
    g = g.max(axis=0)  # [NB, 2]
    zero_blocks = g.sum(axis=1) == 0
    g[zero_blocks, 0] = 1

    aoff = np.concatenate([[0], np.cumsum(g[:, 0])])
    boff = np.concatenate([[0], np.cumsum(g[:, 1])])
    GA, GB = int(aoff[-1]), int(boff[-1])
    G = GA + GB

    # -1 padding goes at each segment tail (the gather ucode trims trailing
    # negatives); the gather-tile buffers are memset once at startup so
    # trimmed lanes read zeros, never uninitialized SBUF
    calls = _segments(g)
    neg_ok = np.zeros((NB, 2), dtype=bool)
    for c in calls:
        if c["tail"] and PAD_NEG:
            neg_ok[c["b"], c["lst"]] = True

    key = ((core * NB + blk) * 2 + half)
    order = np.argsort(key, kind="stable")
    key_sorted = key[order]
    src_sorted = src[order]
    srel_sorted = srel[order]
    seg_starts = np.searchsorted(key_sorted, np.arange(N_CORES * NB * 2))
    seg_ends = np.searchsorted(key_sorted, np.arange(N_CORES * NB * 2), side="right")

    idx_vals = np.zeros((N_CORES, G, 128), dtype=np.int16)
    slot_vals = np.full((N_CORES, G, 128), -1.0, dtype=np.float32)
    for c in range(N_CORES):
        for b in range(NB):
            for h in range(2):
                s, e = seg_starts[(c * NB + b) * 2 + h], seg_ends[(c * NB + b) * 2 + h]
                n = e - s
                ng = int(g[b, h])
                if ng == 0:
                    continue
                g0 = (aoff[b] if h == 0 else GA + boff[b])
                iv = idx_vals[c, g0:g0 + ng].reshape(-1)
                sv = slot_vals[c, g0:g0 + ng].reshape(-1)
                if n:
                    sseg = src_sorted[s:e]
                    iv[:n] = (sseg - HALF * h).astype(np.int16)
                    sv[:n] = srel_sorted[s:e]
                if neg_ok[b, h]:
                    iv[n:] = -1

    # wrapped int16 layout for dma_gather: value (g, q) -> [q%16, 8*g + q//16],
    # replicated across the 8 sixteen-partition stripes
    w = idx_vals.reshape(N_CORES, G, 8, 16).transpose(0, 3, 1, 2).reshape(N_CORES, 16, G * 8)
    idxw = np.tile(w, (1, 8, 1))                       # [C, 128, G*8] int16
    slotw = slot_vals.transpose(0, 2, 1).copy()        # [C, 128, G] f32

    # per-core inverse degrees, [C, 128, NB]: invd[c, p, b] = 1/max(deg, 1)
    deg = np.bincount(dst, minlength=N_NODES).astype(np.float64)
    invd = (1.0 / np.maximum(deg, 1.0)).astype(np.float32)
    invdw = np.zeros((N_CORES, 128, NB), dtype=np.float32)
    for c in range(N_CORES):
        block = np.zeros(NB * BLK, dtype=np.float32)
        block[:NLOC] = invd[c * NLOC:(c + 1) * NLOC]
        invdw[c] = block.reshape(NB, BLK).T

    layout = dict(g=g, aoff=aoff, boff=boff, GA=GA, GB=GB)
    return idxw, slotw, invdw, layout


def _build_program(layout):
    g, aoff, boff = layout["g"], layout["aoff"], layout["boff"]
    GA, GB = layout["GA"], layout["GB"]
    G = GA + GB
    f32 = mybir.dt.float32
    bf16 = mybir.dt.bfloat16

    nc = bacc.Bacc("TRN2", target_bir_lowering=False, debug=False,
                   num_devices=N_CORES, dynamic_dma_scratch_size=SWDGE_SCRATCH,
                   num_swdge_queues=NUM_QUEUES)
    # two separate tensors: dma_gather's ucode mishandles nonzero source-AP
    # offsets on HW, so each int16-addressable half gets its own tensor
    embA = nc.dram_tensor("embA", [HALF, D_FEAT], bf16, kind="ExternalInput").ap()
    embB = nc.dram_tensor("embB", [N_NODES - HALF, D_FEAT], bf16, kind="ExternalInput").ap()
    # slot/iota stay f32: a bf16 (16-bit 2x perf-mode) DVE op locks GpSimd out
    # of the shared SBUF port pair for its whole duration, starving SWDGE
    # descriptor generation -- the kernel's critical path
    iota = nc.dram_tensor("iota", [128, BLK], f32, kind="ExternalInput").ap()
    idxw = nc.dram_tensor("idxw", [128, G * 8], mybir.dt.int16, kind="ExternalInput").ap()
    slotw = nc.dram_tensor("slotw", [128, G], f32, kind="ExternalInput").ap()
    invdw = nc.dram_tensor("invdw", [128, NB], f32, kind="ExternalInput").ap()
    out = nc.dram_tensor("out", [NLOC, D_FEAT], f32, kind="ExternalOutput").ap()

    calls = _segments(g)
    maxg = max(c["ncg"] for c in calls)
    # column ranges in idxw per call; bucket calls into NCHUNK idx tiles
    # (split at call boundaries) so early gathers start before the whole
    # index table has loaded
    cols = []
    for c in calls:
        scol0 = c["g0"] if c["lst"] == 0 else GA + c["g0"]
        cols.append((scol0 * 8, c["ncg"] * 8, scol0))
    total_cols = G * 8
    target = (total_cols + NCHUNK - 1) // NCHUNK
    # calls are not column-ordered (A and B interleave); chunk by column space
    # instead: chunk k covers columns [k*target, (k+1)*target), and each call
    # is assigned to the chunk containing its first column; chunk tiles
    # overlap-load enough columns to cover calls that straddle a boundary.
    chunk_lo = [min(k * target, total_cols) for k in range(NCHUNK)]
    chunk_hi = [min((k + 1) * target, total_cols) for k in range(NCHUNK)]
    call_chunk = []
    for (c0, ncols, _s) in cols:
        k = min(c0 // target, NCHUNK - 1)
        call_chunk.append(k)
        chunk_hi[k] = max(chunk_hi[k], c0 + ncols)

    with tile.TileContext(nc) as tc:
        with (
            tc.tile_pool(name="const", bufs=1) as cpool,
            tc.tile_pool(name="gath", bufs=GBUFS) as gpool,
            tc.tile_pool(name="hbuf", bufs=GBUFS) as hpool,
            tc.tile_pool(name="evict", bufs=4) as epool,
            tc.tile_pool(name="psum", bufs=4, space="PSUM") as ppool,
        ):
            idx_tiles = []
            for k in range(NCHUNK):
                w = chunk_hi[k] - chunk_lo[k]
                t = cpool.tile([128, w], mybir.dt.int16, tag=f"idx{k}")
                nc.sync.dma_start(out=t[:], in_=idxw[:, chunk_lo[k]:chunk_hi[k]])
                idx_tiles.append(t)
                if k == 0:
                    # small constants right after the first idx chunk
                    iota_sb = cpool.tile([128, BLK], f32, tag="iota")
                    nc.sync.dma_start(out=iota_sb[:], in_=iota[:])
                    slot_sb = cpool.tile([128, G], f32, tag="slot")
                    nc.sync.dma_start(out=slot_sb[:], in_=slotw[:])
                    invd_sb = cpool.tile([128, NB], f32, tag="invd")
                    nc.sync.dma_start(out=invd_sb[:], in_=invdw[:])

            srcs = {0: embA, 1: embB}
            # issue order: one gather + one h-build per call, block-major
            call_of = {}
            for k, c in enumerate(calls):
                call_of.setdefault((c["lst"], c["b"]), []).append(k)
            tiles = [None] * len(calls)

            def issue_call(k):
                c = calls[k]
                c0, ncols, scol0 = cols[k]
                ncg = c["ncg"]
                ck = call_chunk[k]
                it = idx_tiles[ck]
                t = gpool.tile([128, maxg * 128], bf16, tag="g")
                nc.gpsimd.dma_gather(
                    out_ap=t[:, :ncg * 128].rearrange("p (n e) -> p n e", e=128),
                    in_ap=srcs[c["lst"]],
                    idxs_ap=it[:, c0 - chunk_lo[ck]:c0 - chunk_lo[ck] + ncols],
                    num_idxs=ncg * 128,
                    num_idxs_reg=ncg * 128,
                    elem_size=D_FEAT,
                    single_packet=(ncg <= 8),
                )
                h = hpool.tile([128, maxg * BLK], bf16, tag="h")
                sv = slot_sb[:, scol0:scol0 + ncg]
                sv3 = sv.unsqueeze(2).to_broadcast([128, ncg, BLK])
                io3 = iota_sb[:].unsqueeze(1).to_broadcast([128, ncg, BLK])
                nc.vector.tensor_tensor(
                    out=h[:, :ncg * BLK].rearrange("p (n e) -> p n e", e=BLK),
                    in0=sv3,
                    in1=io3,
                    op=mybir.AluOpType.is_equal,
                )
                tiles[k] = (t, h)

            for b in range(NB):
                bcalls = call_of.get((0, b), []) + call_of.get((1, b), [])
                for k in bcalls:
                    issue_call(k)
                psum_s = ppool.tile([128, BLK], f32, tag="ps")
                ngrp = sum(calls[k]["ncg"] for k in bcalls)
                i = 0
                for k in bcalls:
                    t, h = tiles[k]
                    for j in range(calls[k]["ncg"]):
                        nc.tensor.matmul(
                            out=psum_s[:],
                            lhsT=h[:, j * BLK:(j + 1) * BLK],
                            rhs=t[:, j * 128:(j + 1) * 128],
                            start=(i == 0), stop=(i == ngrp - 1),
                        )
                        i += 1
                    tiles[k] = None
                ot = epool.tile([128, BLK], f32, tag="ot")
                nc.scalar.activation(
                    out=ot[:], in_=psum_s[:],
                    func=mybir.ActivationFunctionType.Copy,
                    scale=invd_sb[:, b:b + 1],
                )
                rows = min(BLK, NLOC - b * BLK)
                nc.sync.dma_start(out=out[b * BLK:b * BLK + rows, :],
                                  in_=ot[:rows, :])

    # Tile's scheduling pass reorders instructions and round-robins SWDGE
    # completion sems over 8 DMASW lanes in FINAL order.  A sem may only ever
    # be incremented from one SWDGE queue (ring-reclaim correctness), so the
    # queue must be a function of the assigned lane: queue = lane % NUM_QUEUES.
    if NUM_QUEUES > 1:
        from concourse.tile_scheduler import PROC_NAME_TO_IDX
        lane_of = {PROC_NAME_TO_IDX[f"DMASW{i}"]: i for i in range(8)}
        fn = nc.m.functions[0]
        insts = [i for blk_ in fn.blocks for i in blk_.instructions]
        for inst in insts:
            if isinstance(inst, mybir.InstDMAGatherAnt):
                lane = lane_of.get(inst.bass_scheduled_proc)
                assert lane is not None, "gather not on a DMASW lane"
                inst.queue_num = lane % NUM_QUEUES

    nc.compile()
    return nc


def _in_maps(author_emb, src, dst):
    emb = np.ascontiguousarray(np.asarray(author_emb, dtype=np.float32))
    src = np.asarray(src).astype(np.int64)
    dst = np.asarray(dst).astype(np.int64)
    assert emb.shape == (N_NODES, D_FEAT) and src.shape == (N_EDGES,)

    idxw, slotw, invdw, layout = _prepare(src, dst)
    key = (layout["GA"], layout["GB"], layout["g"].tobytes())
    if key not in _cache:
        _cache[key] = _build_program(layout)
    nc = _cache[key]

    import ml_dtypes
    embh = emb.astype(ml_dtypes.bfloat16)
    iota_np = np.broadcast_to(np.arange(BLK, dtype=np.float32), (128, BLK)).astype(np.float32)
    embA = np.ascontiguousarray(embh[:HALF])
    embB = np.ascontiguousarray(embh[HALF:])
    maps = [
        {"embA": embA, "embB": embB, "iota": np.ascontiguousarray(iota_np),
         "idxw": idxw[c], "slotw": slotw[c],
         "invdw": invdw[c]}
        for c in range(N_CORES)
    ]
    return nc, maps


def kernel(author_emb, src, dst, n_nodes):
    nc, maps = _in_maps(author_emb, src, dst)
    res = run_bass_kernel_spmd(nc, maps, list(range(N_CORES)))
    out = np.empty((N_NODES, D_FEAT), dtype=np.float32)
    for c in range(N_CORES):
        out[c * NLOC:(c + 1) * NLOC] = res.results[c]["out"]
    return out
